# revision 18
# baseline (speedup 1.0000x reference)
"""GRU decoder kernel for Trainium2 (8 NeuronCores, data-parallel over batch).

Math (PyTorch GRU, gate order r,z,n), per batch element:
    gx_t = x_t * w_ih + b_ih              (input dim == 1 -> rank-1)
    gh_t = h_{t-1} @ w_hh.T + b_hh
    r = sigmoid(gx_r + gh_r); z = sigmoid(gx_z + gh_z)
    n = tanh(gx_n + b_ih_n + r * (gh_n + b_hh_n))
    h_t = (1-z)*n + z*h_{t-1}
    out = h_T @ fc_w.T + fc_b

Design (measured 4.33 ms on HW vs 4.86 ms staged baseline):
  - H [128, 512]: partitions 0-63 = h-coords for u-batch 0-511,
    partitions 64-127 = v-batch 512-1023; free dim = batch.
  - PE cost on TRN2 is streamed COLUMNS (matmuls do not overlap across
    quadrants in practice), so both batch halves are computed by ONE
    K=128 matmul with a block-diagonal lhsT [[W,0],[0,W]] per gate:
    6 matmuls x N=256 per group-step (hr, xr, nh, hz, xz, xn) instead of
    the baseline's 12 quadrant matmuls.
  - X tile rows: 0-62 = timesteps (UNROLL=63) for u, row 63 = ones;
    64-126 = v, row 127 = ones.  The one-hot lhsT carries the gate
    weight at row q and the gate bias at the ones row, so all biases
    ride into PSUM (no bias operand on the sigmoids/tanh).
  - PSUM accumulation: start=True clears has_written for the WHOLE bank,
    so exactly one matmul per bank per step uses start=True; the rest
    use start=False (has_written=0 -> overwrite, =1 -> accumulate).
  - x-side matmuls (no h dependence) are issued before the h-side ones
    and run during the previous step's elementwise phase; the h-update ->
    hr matmul -> sigR hop is the spine head.
  - Split sigmoids: sig-R on the critical spine, sig-Z deferred (z is
    only needed off-spine).
  - zb-form update keeps only 2 DVE ops after tanh on the spine:
    zb=1-z and p2=z*h run early; then h' = zb*n + p2.
  - Per group-step chain: hr-MM -> sigR -> T1=(nh+bnh)*r (STT) ->
    T2=T1+xn -> tanh -> p1=zb*n -> h'=p1+p2.  Two phase-shifted groups
    share the engines; PSUM banks double-buffered by step parity.
  - Hardware loop runs two 63-step blocks per iteration to halve the
    loop-boundary sync + ACT-table-reload stalls.
"""

import os
import sys

sys.path.insert(0, "/opt/trn_rl_repo")

import numpy as np
from contextlib import ExitStack

HIDDEN = 64
OUT = 256
B = 8192
T = int(os.environ.get("GRU_T", 1024))
DT = os.environ.get("GRU_DT", "f16")
NCORES = 8
BC = B // NCORES          # 1024 batch per core
HB = BC // 2              # 512 batch per partition-half
UNROLL = 63               # timesteps per block (row 63 = ones)
NFULL = T // UNROLL       # full blocks
TAIL = T - NFULL * UNROLL
NBLK = NFULL + (1 if TAIL else 0)
NGROUP = 2                # phase-shifted batch groups per core
HG = HB // NGROUP         # 256 free-dim columns per group
CH = 8                    # XT chunk: steps per broadcast-DMA

_CACHE = {}


def _np16():
    if DT == "bf16":
        import ml_dtypes
        return ml_dtypes.bfloat16
    return np.float16


def _build():
    import concourse.bass as bass
    import concourse.tile as tile
    from concourse import bacc, mybir

    f16 = mybir.dt.bfloat16 if DT == "bf16" else mybir.dt.float16
    f32 = mybir.dt.float32
    AF = mybir.ActivationFunctionType
    OP = mybir.AluOpType

    nc = bacc.Bacc("TRN2", target_bir_lowering=False, debug=False,
                   num_devices=NCORES)

    d_x = nc.dram_tensor("xt", [128, NBLK, HB], f16, kind="ExternalInput").ap()
    d_bdr = nc.dram_tensor("bdr", [128, 128], f16, kind="ExternalInput").ap()
    d_bdz = nc.dram_tensor("bdz", [128, 128], f16, kind="ExternalInput").ap()
    d_bdn = nc.dram_tensor("bdn", [128, 128], f16, kind="ExternalInput").ap()
    d_ohr = nc.dram_tensor("ohr", [128, UNROLL, 128], f16, kind="ExternalInput").ap()
    d_ohz = nc.dram_tensor("ohz", [128, UNROLL, 128], f16, kind="ExternalInput").ap()
    d_ohn = nc.dram_tensor("ohn", [128, UNROLL, 128], f16, kind="ExternalInput").ap()
    d_bnh = nc.dram_tensor("bnh", [128, 1], f32, kind="ExternalInput").ap()
    d_wn = nc.dram_tensor("wn", [128, 1], f32, kind="ExternalInput").ap()
    d_bni = nc.dram_tensor("bni", [128, 1], f32, kind="ExternalInput").ap()
    d_fcw = nc.dram_tensor("fcw", [128, OUT], f16, kind="ExternalInput").ap()
    d_fcb = nc.dram_tensor("fcb", [128, 2], f32, kind="ExternalInput").ap()
    d_out = nc.dram_tensor("out", [OUT, BC], f32, kind="ExternalOutput").ap()

    with tile.TileContext(nc) as tc, ExitStack() as ctx:
        singles = ctx.enter_context(tc.tile_pool(name="singles", bufs=1))
        work = ctx.enter_context(tc.tile_pool(name="work", bufs=4))
        psum = ctx.enter_context(tc.tile_pool(name="psum", bufs=1, space="PSUM"))

        X = singles.tile([128, NBLK, HB], f16)
        BDR = singles.tile([128, 128], f16)
        BDZ = singles.tile([128, 128], f16)
        BDN = singles.tile([128, 128], f16)
        OHR = singles.tile([128, UNROLL, 128], f16)
        OHZ = singles.tile([128, UNROLL, 128], f16)
        OHN = singles.tile([128, UNROLL, 128], f16)
        BNH = singles.tile([128, 1], f32)
        WN = singles.tile([128, 1], f32)
        BNI = singles.tile([128, 1], f32)
        FCW = singles.tile([128, OUT], f16)
        FCB = singles.tile([128, 2], f32)
        H = singles.tile([128, HB], f16)

        for dst, src in ((X, d_x), (BDR, d_bdr), (BDZ, d_bdz), (BDN, d_bdn),
                         (OHR, d_ohr), (OHZ, d_ohz), (OHN, d_ohn),
                         (BNH, d_bnh), (WN, d_wn), (BNI, d_bni),
                         (FCW, d_fcw), (FCB, d_fcb)):
            nc.gpsimd.dma_start(dst[:], src[:])
        nc.vector.memset(H[:], 0.0)

        # HAM warmup: back-to-back matmuls lift the PE clock gate to 8/8.
        warm = psum.tile([128, HB], f32, tag="RZ00", name="warm")
        for _ in range(20):
            nc.tensor.matmul(warm[:, 0:HG], BDR[:], H[:, 0:HG],
                             start=True, stop=True)

        def alloc_banks(g, par):
            RZ = psum.tile([128, HB], f32, tag=f"RZ{g}{par}", name="RZ")
            NHX = psum.tile([128, HB], f32, tag=f"NHX{g}{par}", name="NHX")
            return (RZ, NHX)

        def mms_x(q, blk, g, banks):
            # x-side matmuls: no h dependence, run during the previous
            # step's elementwise phase.  One start=True per bank (clears
            # has_written bank-wide); later matmuls use start=False.
            mm = nc.tensor.matmul
            gsl = slice(g * HG, (g + 1) * HG)
            xq = X[:, blk, gsl]
            RZ, NHX = banks
            mm(RZ[:, 0:HG], OHR[:, q, :], xq, start=True, stop=False)
            mm(RZ[:, HG:HB], OHZ[:, q, :], xq, start=False, stop=False,
               skip_group_check=True)
            mm(NHX[:, HG:HB], OHN[:, q, :], xq, start=True, stop=True)

        def mms_h(g, banks):
            # h-side matmuls: wait on this group's previous h-update; hr
            # first (it gates sigR, the chain head)
            mm = nc.tensor.matmul
            gsl = slice(g * HG, (g + 1) * HG)
            h = H[:, gsl]
            RZ, NHX = banks
            mm(RZ[:, 0:HG], BDR[:], h, start=False, stop=True)
            mm(NHX[:, 0:HG], BDN[:], h, start=False, stop=True,
               skip_group_check=True)
            mm(RZ[:, HG:HB], BDZ[:], h, start=False, stop=True)

        def sigr_op(g, banks):
            SR = work.tile([128, HG], f16, tag=f"SR{g}", name="SR")
            nc.scalar.activation(SR[:], banks[0][:, 0:HG], AF.Sigmoid)
            return SR

        def sigz_op(g, banks):
            SZ = work.tile([128, HG], f16, tag=f"SZ{g}", name="SZ")
            nc.scalar.activation(SZ[:], banks[0][:, HG:HB], AF.Sigmoid)
            return SZ

        def t1_op(g, banks, SR):
            T1 = work.tile([128, HG], f16, tag=f"T1{g}", name="T1")
            nc.vector.scalar_tensor_tensor(T1[:], banks[1][:, 0:HG], BNH[:],
                                           SR[:], op0=OP.add, op1=OP.mult)
            return T1

        def t2_op(g, banks, T1):
            T2 = work.tile([128, HG], f16, tag=f"T2{g}", name="T2")
            nc.vector.tensor_add(T2[:], T1[:], banks[1][:, HG:HB])
            return T2

        def tanh_op(g, T2):
            NN = work.tile([128, HG], f16, tag=f"NN{g}", name="NN")
            nc.scalar.activation(NN[:], T2[:], AF.Tanh)
            return NN

        def zb_op(g, SZ):
            ZB = work.tile([128, HG], f16, tag=f"ZB{g}", name="ZB")
            nc.vector.tensor_scalar(ZB[:], SZ[:], -1.0, 1.0,
                                    op0=OP.mult, op1=OP.add)
            return ZB

        def p2_op(g, SZ):
            P2 = work.tile([128, HG], f16, tag=f"P2{g}", name="P2")
            nc.vector.tensor_mul(P2[:], SZ[:], H[:, g * HG:(g + 1) * HG])
            return P2

        def p1_op(g, ZB, NN):
            P1 = work.tile([128, HG], f16, tag=f"P1{g}", name="P1")
            nc.vector.tensor_mul(P1[:], ZB[:], NN[:])
            return P1

        def upd_op(g, P1, P2):
            nc.vector.tensor_add(H[:, g * HG:(g + 1) * HG], P1[:], P2[:])

        def half_step(gA, pendA, gB, banksB):
            """Finish group gA's step (tanh..h'-update) interleaved with
            group gB's first half-chain (sigR, sigZ, T1, T2)."""
            if pendA is not None:
                ZBa, P2a, T2a = pendA
                NNa = tanh_op(gA, T2a)
                SRb = sigr_op(gB, banksB)
                P1a = p1_op(gA, ZBa, NNa)
                T1b = t1_op(gB, banksB, SRb)
                upd_op(gA, P1a, P2a)
                SZb = sigz_op(gB, banksB)
                T2b = t2_op(gB, banksB, T1b)
                ZBb = zb_op(gB, SZb)
                P2b = p2_op(gB, SZb)
            else:
                SRb = sigr_op(gB, banksB)
                SZb = sigz_op(gB, banksB)
                T1b = t1_op(gB, banksB, SRb)
                T2b = t2_op(gB, banksB, T1b)
                ZBb = zb_op(gB, SZb)
                P2b = p2_op(gB, SZb)
            return (ZBb, P2b, T2b)

        def flush(g, pend):
            ZB, P2, T2 = pend
            NN = tanh_op(g, T2)
            P1 = p1_op(g, ZB, NN)
            upd_op(g, P1, P2)

        def body(blk, nstep, pend1=None, drain=True):
            if isinstance(blk, int):
                blk = slice(blk, blk + 1)
            for q in range(nstep):
                par = q % 2
                b0 = alloc_banks(0, par)
                b1 = alloc_banks(1, par)
                # both groups' x-matmuls first: they have no h dependence,
                # so the PE FIFO reaches h0 with only ready work ahead of
                # it and each group's h-burst unblocks at its own h-update
                mms_x(q, blk, 0, b0)
                mms_x(q, blk, 1, b1)
                mms_h(0, b0)
                pend0 = half_step(1, pend1, 0, b0)
                mms_h(1, b1)
                pend1 = half_step(0, pend0, 1, b1)
            if drain:
                flush(1, pend1)
                return None
            return pend1

        if NFULL <= 1 or os.environ.get("GRU_NOHWLOOP"):
            for blk in range(NFULL):
                body(blk, UNROLL)
        elif NFULL % 4 == 0 and not os.environ.get("GRU_LOOP1"):
            # four blocks per hardware-loop iteration: cuts the ~5us
            # loop-boundary sync + ACT table reload stalls 4x
            with tc.For_i(0, NFULL, 4,
                          hint_engines=(mybir.EngineType.PE,)) as i:
                # carry the 2-group pipeline across the 4 blocks; drain
                # only at the iteration boundary (cross-iteration tile
                # references are not allowed in the hardware loop)
                p = body(bass.ds(i, 1), UNROLL, None, drain=False)
                p = body(bass.ds(i + 1, 1), UNROLL, p, drain=False)
                p = body(bass.ds(i + 2, 1), UNROLL, p, drain=False)
                body(bass.ds(i + 3, 1), UNROLL, p, drain=True)
        elif NFULL % 2 == 0 and not os.environ.get("GRU_LOOP1"):
            # two blocks per hardware-loop iteration: halves the ~5us
            # loop-boundary sync + ACT table reload stalls
            with tc.For_i(0, NFULL, 2,
                          hint_engines=(mybir.EngineType.PE,)) as i:
                body(bass.ds(i, 1), UNROLL)
                body(bass.ds(i + 1, 1), UNROLL)
        else:
            with tc.For_i(0, NFULL, 1,
                          hint_engines=(mybir.EngineType.PE,)) as i:
                body(bass.ds(i, 1), UNROLL)
        if TAIL:
            body(NFULL, TAIL)

        # Final FC: out[o, b] = sum_k fc_w[o, k] h[b, k] + fc_b[o]
        for oh in range(2):
            osl = slice(oh * 128, (oh + 1) * 128)
            fc_u = psum.tile([128, HB], f32, tag="RZ00")
            fc_v = psum.tile([128, HB], f32, tag="RZ10")
            nc.tensor.matmul(fc_u[:], FCW[0:64, osl], H[0:64, :],
                             start=True, stop=True, tile_position=(0, 0))
            nc.tensor.matmul(fc_v[:], FCW[64:128, osl], H[64:128, :],
                             start=True, stop=True, tile_position=(64, 0))
            Ou = work.tile([128, HB], f32, tag="Ou")
            Ov = work.tile([128, HB], f32, tag="Ov")
            nc.scalar.activation(Ou[:], fc_u[:], AF.Identity,
                                 bias=FCB[:, oh:oh + 1])
            nc.scalar.activation(Ov[:], fc_v[:], AF.Identity,
                                 bias=FCB[:, oh:oh + 1])
            nc.gpsimd.dma_start(d_out[osl, 0:HB], Ou[:])
            nc.gpsimd.dma_start(d_out[osl, HB:BC], Ov[:])

    nc.compile()
    return nc


def _host_inputs(x, w_ih, w_hh, b_ih, b_hh, fc_w, fc_b):
    """Build the per-core in_maps (numpy, laid out exactly as SBUF tiles)."""
    f16 = _np16()
    f32 = np.float32
    x = np.asarray(x, f32)
    w_ih = np.asarray(w_ih, f32)[:, 0]                # [192]
    w_hh = np.asarray(w_hh, f32)                      # [192, 64]
    b_ih = np.asarray(b_ih, f32)
    b_hh = np.asarray(b_hh, f32)
    fc_w = np.asarray(fc_w, f32)
    fc_b = np.asarray(fc_b, f32)

    def blockdiag(seg):
        wt = w_hh[seg, :].T                            # [64 k, 64 m]
        bd = np.zeros((128, 128), f32)
        bd[0:64, 0:64] = wt
        bd[64:128, 64:128] = wt
        return bd.astype(f16)

    def oh(seg, bias):
        w = w_ih[seg]                                  # [64]
        o = np.zeros((128, UNROLL, 128), f32)
        for q in range(UNROLL):
            o[q, q, 0:64] = w
            o[64 + q, q, 64:128] = w
        o[63, :, 0:64] = bias
        o[127, :, 64:128] = bias
        return o.astype(f16)

    def col(v):
        return np.tile(v.reshape(-1, 1), (2, 1)).astype(f32)   # [128, 1]

    shared = {
        "bdr": blockdiag(slice(0, 64)),
        "bdz": blockdiag(slice(64, 128)),
        "bdn": blockdiag(slice(128, 192)),
        "ohr": oh(slice(0, 64), b_ih[0:64] + b_hh[0:64]),
        "ohz": oh(slice(64, 128), b_ih[64:128] + b_hh[64:128]),
        "ohn": oh(slice(128, 192), b_ih[128:192]),
        "bnh": col(b_hh[128:192]),
        "wn": col(w_ih[128:192]),
        "bni": col(b_ih[128:192]),
        "fcw": np.vstack([fc_w.T, fc_w.T]).astype(f16),  # [128, 256]
        "fcb": np.stack([fc_b[0:128], fc_b[128:256]], 1).astype(f32),
    }

    in_maps = []
    for c in range(NCORES):
        xs = x[c * BC:(c + 1) * BC, :T, 0]            # [BC b, T t]
        Xh = np.zeros((128, NBLK, HB), f32)
        for blk in range(NBLK):
            t0 = blk * UNROLL
            nq = min(UNROLL, T - t0)
            Xh[0:nq, blk, :] = xs[0:HB, t0:t0 + nq].T
            Xh[64:64 + nq, blk, :] = xs[HB:BC, t0:t0 + nq].T
        Xh[63, :, :] = 1.0
        Xh[127, :, :] = 1.0
        m = dict(shared)
        m["xt"] = np.ascontiguousarray(Xh).astype(f16)
        in_maps.append(m)
    return in_maps


def _run(in_maps, trace=False):
    from concourse import bass_utils
    if "nc" not in _CACHE:
        _CACHE["nc"] = _build()
    nc = _CACHE["nc"]
    res = bass_utils.run_bass_kernel_spmd(
        nc, in_maps, core_ids=list(range(NCORES)), trace=trace)
    return res


def kernel(**inputs):
    in_maps = _host_inputs(**inputs)
    res = _run(in_maps, trace=False)
    out = np.empty([B, OUT], np.float32)
    for c in range(NCORES):
        out[c * BC:(c + 1) * BC, :] = res.results[c]["out"].T
    return out


# revision 20
# speedup vs baseline: 1.0027x; 1.0027x over previous
"""GRU decoder kernel for Trainium2 (8 NeuronCores, data-parallel over batch).

Math (PyTorch GRU, gate order r,z,n), per batch element:
    gx_t = x_t * w_ih + b_ih              (input dim == 1 -> rank-1)
    gh_t = h_{t-1} @ w_hh.T + b_hh
    r = sigmoid(gx_r + gh_r); z = sigmoid(gx_z + gh_z)
    n = tanh(gx_n + b_ih_n + r * (gh_n + b_hh_n))
    h_t = (1-z)*n + z*h_{t-1}
    out = h_T @ fc_w.T + fc_b

Design (measured 4.30 ms on HW vs 4.86 ms staged baseline):
  - H [128, 512]: partitions 0-63 = h-coords for u-batch 0-511,
    partitions 64-127 = v-batch 512-1023; free dim = batch.
  - PE cost on TRN2 is streamed COLUMNS (matmuls do not overlap across
    quadrants in practice), so both batch halves are computed by ONE
    K=128 matmul with a block-diagonal lhsT [[W,0],[0,W]] per gate:
    6 matmuls x N=256 per group-step (hr, xr, nh, hz, xz, xn) instead of
    the baseline's 12 quadrant matmuls.
  - X tile rows: 0-62 = timesteps (UNROLL=63) for u, row 63 = ones;
    64-126 = v, row 127 = ones.  The one-hot lhsT carries the gate
    weight at row q and the gate bias at the ones row, so all biases
    ride into PSUM (no bias operand on the sigmoids/tanh).
  - PSUM accumulation: start=True clears has_written for the WHOLE bank,
    so exactly one matmul per bank per step uses start=True; the rest
    use start=False (has_written=0 -> overwrite, =1 -> accumulate).
  - x-side matmuls (no h dependence) are issued before the h-side ones
    and run during the previous step's elementwise phase; the h-update ->
    hr matmul -> sigR hop is the spine head.
  - Split sigmoids: sig-R on the critical spine, sig-Z deferred (z is
    only needed off-spine).
  - zb-form update keeps only 2 DVE ops after tanh on the spine:
    zb=1-z and p2=z*h run early; then h' = zb*n + p2.
  - Per group-step chain: hr-MM -> sigR -> T1=(nh+bnh)*r (STT) ->
    T2=T1+xn -> tanh -> p1=zb*n -> h'=p1+p2.  Two phase-shifted groups
    share the engines; PSUM banks double-buffered by step parity.
  - Hardware loop runs four 63-step blocks per iteration (cuts the ~5us
    loop-boundary sync + ACT-table-reload stalls 4x), with the 2-group
    pipeline carried across the blocks inside an iteration.
"""

import os
import sys

sys.path.insert(0, "/opt/trn_rl_repo")

import numpy as np
from contextlib import ExitStack

HIDDEN = 64
OUT = 256
B = 8192
T = int(os.environ.get("GRU_T", 1024))
DT = os.environ.get("GRU_DT", "f16")
NCORES = 8
BC = B // NCORES          # 1024 batch per core
HB = BC // 2              # 512 batch per partition-half
UNROLL = 63               # timesteps per block (row 63 = ones)
NFULL = T // UNROLL       # full blocks
TAIL = T - NFULL * UNROLL
NBLK = NFULL + (1 if TAIL else 0)
NGROUP = 2                # phase-shifted batch groups per core
HG = HB // NGROUP         # 256 free-dim columns per group
CH = 8                    # XT chunk: steps per broadcast-DMA

_CACHE = {}


def _np16():
    if DT == "bf16":
        import ml_dtypes
        return ml_dtypes.bfloat16
    return np.float16


def _build():
    import concourse.bass as bass
    import concourse.tile as tile
    from concourse import bacc, mybir

    f16 = mybir.dt.bfloat16 if DT == "bf16" else mybir.dt.float16
    f32 = mybir.dt.float32
    AF = mybir.ActivationFunctionType
    OP = mybir.AluOpType

    nc = bacc.Bacc("TRN2", target_bir_lowering=False, debug=False,
                   num_devices=NCORES)

    d_x = nc.dram_tensor("xt", [128, NBLK, HB], f16, kind="ExternalInput").ap()
    d_bdr = nc.dram_tensor("bdr", [128, 128], f16, kind="ExternalInput").ap()
    d_bdz = nc.dram_tensor("bdz", [128, 128], f16, kind="ExternalInput").ap()
    d_bdn = nc.dram_tensor("bdn", [128, 128], f16, kind="ExternalInput").ap()
    d_ohr = nc.dram_tensor("ohr", [128, UNROLL, 128], f16, kind="ExternalInput").ap()
    d_ohz = nc.dram_tensor("ohz", [128, UNROLL, 128], f16, kind="ExternalInput").ap()
    d_ohn = nc.dram_tensor("ohn", [128, UNROLL, 128], f16, kind="ExternalInput").ap()
    d_bnh = nc.dram_tensor("bnh", [128, 1], f32, kind="ExternalInput").ap()
    d_wn = nc.dram_tensor("wn", [128, 1], f32, kind="ExternalInput").ap()
    d_bni = nc.dram_tensor("bni", [128, 1], f32, kind="ExternalInput").ap()
    d_fcw = nc.dram_tensor("fcw", [128, OUT], f16, kind="ExternalInput").ap()
    d_fcb = nc.dram_tensor("fcb", [128, 2], f32, kind="ExternalInput").ap()
    d_out = nc.dram_tensor("out", [OUT, BC], f32, kind="ExternalOutput").ap()

    with tile.TileContext(nc) as tc, ExitStack() as ctx:
        singles = ctx.enter_context(tc.tile_pool(name="singles", bufs=1))
        work = ctx.enter_context(tc.tile_pool(name="work", bufs=4))
        psum = ctx.enter_context(tc.tile_pool(name="psum", bufs=1, space="PSUM"))

        X = singles.tile([128, NBLK, HB], f16)
        BDR = singles.tile([128, 128], f16)
        BDZ = singles.tile([128, 128], f16)
        BDN = singles.tile([128, 128], f16)
        OHR = singles.tile([128, UNROLL, 128], f16)
        OHZ = singles.tile([128, UNROLL, 128], f16)
        OHN = singles.tile([128, UNROLL, 128], f16)
        BNH = singles.tile([128, 1], f32)
        WN = singles.tile([128, 1], f32)
        BNI = singles.tile([128, 1], f32)
        FCW = singles.tile([128, OUT], f16)
        FCB = singles.tile([128, 2], f32)
        H = singles.tile([128, HB], f16)

        for dst, src in ((X, d_x), (BDR, d_bdr), (BDZ, d_bdz), (BDN, d_bdn),
                         (OHR, d_ohr), (OHZ, d_ohz), (OHN, d_ohn),
                         (BNH, d_bnh), (WN, d_wn), (BNI, d_bni),
                         (FCW, d_fcw), (FCB, d_fcb)):
            nc.gpsimd.dma_start(dst[:], src[:])
        nc.vector.memset(H[:], 0.0)

        # HAM warmup: back-to-back matmuls lift the PE clock gate to 8/8.
        warm = psum.tile([128, HB], f32, tag="RZ00", name="warm")
        for _ in range(20):
            nc.tensor.matmul(warm[:, 0:HG], BDR[:], H[:, 0:HG],
                             start=True, stop=True)

        def alloc_banks(g, par):
            RZ = psum.tile([128, HB], f32, tag=f"RZ{g}{par}", name="RZ")
            NHX = psum.tile([128, HB], f32, tag=f"NHX{g}{par}", name="NHX")
            return (RZ, NHX)

        def mms_x(q, blk, g, banks):
            # x-side matmuls: no h dependence, run during the previous
            # step's elementwise phase.  One start=True per bank (clears
            # has_written bank-wide); later matmuls use start=False.
            mm = nc.tensor.matmul
            gsl = slice(g * HG, (g + 1) * HG)
            xq = X[:, blk, gsl]
            RZ, NHX = banks
            mm(RZ[:, 0:HG], OHR[:, q, :], xq, start=True, stop=False)
            mm(RZ[:, HG:HB], OHZ[:, q, :], xq, start=False, stop=False,
               skip_group_check=True)
            mm(NHX[:, HG:HB], OHN[:, q, :], xq, start=True, stop=True)

        def mms_h(g, banks):
            # h-side matmuls: wait on this group's previous h-update; hr
            # first (it gates sigR, the chain head)
            mm = nc.tensor.matmul
            gsl = slice(g * HG, (g + 1) * HG)
            h = H[:, gsl]
            RZ, NHX = banks
            mm(RZ[:, 0:HG], BDR[:], h, start=False, stop=True)
            mm(NHX[:, 0:HG], BDN[:], h, start=False, stop=True,
               skip_group_check=True)
            mm(RZ[:, HG:HB], BDZ[:], h, start=False, stop=True)

        def sigr_op(g, banks):
            SR = work.tile([128, HG], f16, tag=f"SR{g}", name="SR")
            nc.scalar.activation(SR[:], banks[0][:, 0:HG], AF.Sigmoid)
            return SR

        def sigz_op(g, banks):
            SZ = work.tile([128, HG], f16, tag=f"SZ{g}", name="SZ")
            nc.scalar.activation(SZ[:], banks[0][:, HG:HB], AF.Sigmoid)
            return SZ

        def t1_op(g, banks, SR):
            T1 = work.tile([128, HG], f16, tag=f"T1{g}", name="T1")
            nc.vector.scalar_tensor_tensor(T1[:], banks[1][:, 0:HG], BNH[:],
                                           SR[:], op0=OP.add, op1=OP.mult)
            return T1

        def t2_op(g, banks, T1):
            T2 = work.tile([128, HG], f16, tag=f"T2{g}", name="T2")
            nc.vector.tensor_add(T2[:], T1[:], banks[1][:, HG:HB])
            return T2

        def tanh_op(g, T2):
            NN = work.tile([128, HG], f16, tag=f"NN{g}", name="NN")
            nc.scalar.activation(NN[:], T2[:], AF.Tanh)
            return NN

        def zb_op(g, SZ):
            ZB = work.tile([128, HG], f16, tag=f"ZB{g}", name="ZB")
            nc.vector.tensor_scalar(ZB[:], SZ[:], -1.0, 1.0,
                                    op0=OP.mult, op1=OP.add)
            return ZB

        def p2_op(g, SZ):
            P2 = work.tile([128, HG], f16, tag=f"P2{g}", name="P2")
            nc.vector.tensor_mul(P2[:], SZ[:], H[:, g * HG:(g + 1) * HG])
            return P2

        def p1_op(g, ZB, NN):
            P1 = work.tile([128, HG], f16, tag=f"P1{g}", name="P1")
            nc.vector.tensor_mul(P1[:], ZB[:], NN[:])
            return P1

        def upd_op(g, P1, P2):
            nc.vector.tensor_add(H[:, g * HG:(g + 1) * HG], P1[:], P2[:])

        def half_step(gA, pendA, gB, banksB):
            """Finish group gA's step (tanh..h'-update) interleaved with
            group gB's first half-chain (sigR, sigZ, T1, T2)."""
            if pendA is not None:
                ZBa, P2a, T2a = pendA
                NNa = tanh_op(gA, T2a)
                SRb = sigr_op(gB, banksB)
                P1a = p1_op(gA, ZBa, NNa)
                T1b = t1_op(gB, banksB, SRb)
                upd_op(gA, P1a, P2a)
                SZb = sigz_op(gB, banksB)
                T2b = t2_op(gB, banksB, T1b)
                ZBb = zb_op(gB, SZb)
                P2b = p2_op(gB, SZb)
            else:
                SRb = sigr_op(gB, banksB)
                SZb = sigz_op(gB, banksB)
                T1b = t1_op(gB, banksB, SRb)
                T2b = t2_op(gB, banksB, T1b)
                ZBb = zb_op(gB, SZb)
                P2b = p2_op(gB, SZb)
            return (ZBb, P2b, T2b)

        def flush(g, pend):
            ZB, P2, T2 = pend
            NN = tanh_op(g, T2)
            P1 = p1_op(g, ZB, NN)
            upd_op(g, P1, P2)

        def body(blk, nstep, pend1=None, drain=True):
            if isinstance(blk, int):
                blk = slice(blk, blk + 1)
            for q in range(nstep):
                par = q % 2
                b0 = alloc_banks(0, par)
                b1 = alloc_banks(1, par)
                # both groups' x-matmuls first: they have no h dependence,
                # so the PE FIFO reaches h0 with only ready work ahead of
                # it and each group's h-burst unblocks at its own h-update
                mms_x(q, blk, 0, b0)
                mms_x(q, blk, 1, b1)
                mms_h(0, b0)
                pend0 = half_step(1, pend1, 0, b0)
                mms_h(1, b1)
                pend1 = half_step(0, pend0, 1, b1)
            if drain:
                flush(1, pend1)
                return None
            return pend1

        if NFULL <= 1 or os.environ.get("GRU_NOHWLOOP"):
            for blk in range(NFULL):
                body(blk, UNROLL)
        elif NFULL % 4 == 0 and not os.environ.get("GRU_LOOP1"):
            # four blocks per hardware-loop iteration: cuts the ~5us
            # loop-boundary sync + ACT table reload stalls 4x
            with tc.For_i(0, NFULL, 4,
                          hint_engines=(mybir.EngineType.PE,)) as i:
                # carry the 2-group pipeline across the 4 blocks; drain
                # only at the iteration boundary (cross-iteration tile
                # references are not allowed in the hardware loop)
                p = body(bass.ds(i, 1), UNROLL, None, drain=False)
                p = body(bass.ds(i + 1, 1), UNROLL, p, drain=False)
                p = body(bass.ds(i + 2, 1), UNROLL, p, drain=False)
                body(bass.ds(i + 3, 1), UNROLL, p, drain=True)
        elif NFULL % 2 == 0 and not os.environ.get("GRU_LOOP1"):
            # two blocks per hardware-loop iteration: halves the ~5us
            # loop-boundary sync + ACT table reload stalls
            with tc.For_i(0, NFULL, 2,
                          hint_engines=(mybir.EngineType.PE,)) as i:
                body(bass.ds(i, 1), UNROLL)
                body(bass.ds(i + 1, 1), UNROLL)
        else:
            with tc.For_i(0, NFULL, 1,
                          hint_engines=(mybir.EngineType.PE,)) as i:
                body(bass.ds(i, 1), UNROLL)
        if TAIL:
            body(NFULL, TAIL)

        # Final FC: out[o, b] = sum_k fc_w[o, k] h[b, k] + fc_b[o]
        for oh in range(2):
            osl = slice(oh * 128, (oh + 1) * 128)
            fc_u = psum.tile([128, HB], f32, tag="RZ00")
            fc_v = psum.tile([128, HB], f32, tag="RZ10")
            nc.tensor.matmul(fc_u[:], FCW[0:64, osl], H[0:64, :],
                             start=True, stop=True, tile_position=(0, 0))
            nc.tensor.matmul(fc_v[:], FCW[64:128, osl], H[64:128, :],
                             start=True, stop=True, tile_position=(64, 0))
            Ou = work.tile([128, HB], f32, tag="Ou")
            Ov = work.tile([128, HB], f32, tag="Ov")
            nc.scalar.activation(Ou[:], fc_u[:], AF.Identity,
                                 bias=FCB[:, oh:oh + 1])
            nc.scalar.activation(Ov[:], fc_v[:], AF.Identity,
                                 bias=FCB[:, oh:oh + 1])
            nc.gpsimd.dma_start(d_out[osl, 0:HB], Ou[:])
            nc.gpsimd.dma_start(d_out[osl, HB:BC], Ov[:])

    nc.compile()
    return nc


def _host_inputs(x, w_ih, w_hh, b_ih, b_hh, fc_w, fc_b):
    """Build the per-core in_maps (numpy, laid out exactly as SBUF tiles)."""
    f16 = _np16()
    f32 = np.float32
    x = np.asarray(x, f32)
    w_ih = np.asarray(w_ih, f32)[:, 0]                # [192]
    w_hh = np.asarray(w_hh, f32)                      # [192, 64]
    b_ih = np.asarray(b_ih, f32)
    b_hh = np.asarray(b_hh, f32)
    fc_w = np.asarray(fc_w, f32)
    fc_b = np.asarray(fc_b, f32)

    def blockdiag(seg):
        wt = w_hh[seg, :].T                            # [64 k, 64 m]
        bd = np.zeros((128, 128), f32)
        bd[0:64, 0:64] = wt
        bd[64:128, 64:128] = wt
        return bd.astype(f16)

    def oh(seg, bias):
        w = w_ih[seg]                                  # [64]
        o = np.zeros((128, UNROLL, 128), f32)
        for q in range(UNROLL):
            o[q, q, 0:64] = w
            o[64 + q, q, 64:128] = w
        o[63, :, 0:64] = bias
        o[127, :, 64:128] = bias
        return o.astype(f16)

    def col(v):
        return np.tile(v.reshape(-1, 1), (2, 1)).astype(f32)   # [128, 1]

    shared = {
        "bdr": blockdiag(slice(0, 64)),
        "bdz": blockdiag(slice(64, 128)),
        "bdn": blockdiag(slice(128, 192)),
        "ohr": oh(slice(0, 64), b_ih[0:64] + b_hh[0:64]),
        "ohz": oh(slice(64, 128), b_ih[64:128] + b_hh[64:128]),
        "ohn": oh(slice(128, 192), b_ih[128:192]),
        "bnh": col(b_hh[128:192]),
        "wn": col(w_ih[128:192]),
        "bni": col(b_ih[128:192]),
        "fcw": np.vstack([fc_w.T, fc_w.T]).astype(f16),  # [128, 256]
        "fcb": np.stack([fc_b[0:128], fc_b[128:256]], 1).astype(f32),
    }

    in_maps = []
    for c in range(NCORES):
        xs = x[c * BC:(c + 1) * BC, :T, 0]            # [BC b, T t]
        Xh = np.zeros((128, NBLK, HB), f32)
        for blk in range(NBLK):
            t0 = blk * UNROLL
            nq = min(UNROLL, T - t0)
            Xh[0:nq, blk, :] = xs[0:HB, t0:t0 + nq].T
            Xh[64:64 + nq, blk, :] = xs[HB:BC, t0:t0 + nq].T
        Xh[63, :, :] = 1.0
        Xh[127, :, :] = 1.0
        m = dict(shared)
        m["xt"] = np.ascontiguousarray(Xh).astype(f16)
        in_maps.append(m)
    return in_maps


def _run(in_maps, trace=False):
    from concourse import bass_utils
    if "nc" not in _CACHE:
        _CACHE["nc"] = _build()
    nc = _CACHE["nc"]
    res = bass_utils.run_bass_kernel_spmd(
        nc, in_maps, core_ids=list(range(NCORES)), trace=trace)
    return res


def kernel(**inputs):
    in_maps = _host_inputs(**inputs)
    res = _run(in_maps, trace=False)
    out = np.empty([B, OUT], np.float32)
    for c in range(NCORES):
        out[c * BC:(c + 1) * BC, :] = res.results[c]["out"].T
    return out
